# revision 41
# baseline (speedup 1.0000x reference)
"""GPRGNN on 8 Trainium2 NeuronCores (Bass/Tile SPMD kernel).

Model: h = relu(x@W1+b1)@W2+b2; 10 hops of GCN-normalized propagation
(A_hat = D^-1/2 (A+I) D^-1/2) accumulated with GPR coefficients temp[k];
log_softmax output.

Strategy (nodes sharded by dst across 8 cores):
  - Reformulate with u_k = dinv * h_k:  u_{k+1} = dinv^2 * (S u_k) where S is
    the plain (self-loop-augmented) adjacency sum; hidden = (1/dinv) * sum_k
    temp[k] u_k.
  - Each hop: each core dma_gathers u rows (bf16, 256B rows, edge slots) for
    its edges from an HBM-resident full copy of u (int16 indices within 4
    chunks of 25k nodes), then segment-sums via PE matmuls with the one-hot
    dst-indicator as the FWL stationary operand: stationary M [128 slots,
    128 dsts] fp8, moving gathered [128 slots, 64 feats] bf16, accumulating
    into PSUM [128 dsts, 64 feats] (row orientation; no transpose-back).
  - M tiles are generated on-chip by the Vector engine from a compact int
    col-index stream: M[s, j] = (cidx[s] == iota[j]) via a broadcast
    tensor_tensor is_equal (pads use cidx = -1).
  - New u shard rows are scaled (dinv^2) and staged, then AllGathered for
    the next hop.
  - Edge structure is identical across cores (SPMD single program) by
    padding each (chunk, 128-dst-block) edge block to the max tile count
    over cores; pad slots gather row 0 with cidx -1.
"""

import math
import os
import sys

import numpy as np

sys.path.insert(0, "/opt/trn_rl_repo")

import ml_dtypes

N = 100000
F_IN = 512
F_HID = 256
F_OUT = 64
K_HOPS = 10
P_CORES = 8
NSH = N // P_CORES  # 12500 nodes per core (dst shard)
NCHUNK = 25000  # gather source chunk (int16 index range)
N_CHUNKS = N // NCHUNK  # 4
N_BLK = (NSH + 127) // 128  # 98 dst blocks of 128 per core (last has 84)
NSH_PAD = N_BLK * 128  # 12544 staged rows per shard
CHUNK_ROWS = 2 * NSH_PAD  # 25088 staged rows per gather chunk
WIN = 512  # MLP column window
N_WIN = (NSH + WIN - 1) // WIN  # 25 (last window 212 cols)
BLK_PER_GRP = 8  # dst blocks per PSUM group (one 2KB bank)
N_GRP = (N_BLK + BLK_PER_GRP - 1) // BLK_PER_GRP  # 13 (last group 2 blocks)

T_SUB = 24  # max gather/matmul tiles per pipelined sub-unit

BF16 = ml_dtypes.bfloat16
FP8 = ml_dtypes.float8_e4m3


def _win_cols(w):
    return min(WIN, NSH - w * WIN)


def _blk_cols(b):
    return min(128, NSH - b * 128)


def _grp_blocks(g):
    return list(range(g * BLK_PER_GRP, min(N_BLK, (g + 1) * BLK_PER_GRP)))


def _prep_host(x, edge_index, W1, b1, W2, b2, temp):
    """All numpy preprocessing: normalization, edge sorting/packing, layouts."""
    src = np.concatenate([edge_index[0].astype(np.int64), np.arange(N, dtype=np.int64)])
    dst = np.concatenate([edge_index[1].astype(np.int64), np.arange(N, dtype=np.int64)])
    deg = np.bincount(dst, minlength=N).astype(np.float64)
    dinv = 1.0 / np.sqrt(deg)  # deg >= 1 (self loops)
    dinv2 = (dinv * dinv).astype(np.float32)
    dinv = dinv.astype(np.float32)

    core = dst // NSH
    ldst = dst - core * NSH
    blk = ldst // 128
    col = ldst % 128
    chunk = src // NCHUNK

    # staged row index of src within its chunk
    half = (src // NSH) % 2
    lsrc = src % NSH
    srow = (half * NSH_PAD + (lsrc % 128) * N_BLK + lsrc // 128).astype(np.int16)

    order = np.lexsort((blk, chunk, core))
    core_s = core[order]
    chunk_s = chunk[order]
    blk_s = blk[order]
    col_s = col[order]
    srow_s = srow[order]

    counts = np.zeros((P_CORES, N_CHUNKS, N_BLK), dtype=np.int64)
    np.add.at(counts, (core_s, chunk_s, blk_s), 1)
    tiles_cb = (counts.max(axis=0) + 127) // 128  # [chunk, blk]
    assert (counts > 0).all(), "some (core,chunk,blk) has zero edges"

    # edge range starts per (core, chunk, blk)
    key = (core_s * N_CHUNKS + chunk_s) * N_BLK + blk_s
    starts = np.searchsorted(key, np.arange(P_CORES * N_CHUNKS * N_BLK))
    ends = np.append(starts[1:], len(key))

    # slot stream order: for g, for c, for b in grp, for t: 128 slots
    total_tiles = int(tiles_cb.sum())
    total_slots = total_tiles * 128

    # per-(g,c) offsets and tile meta (identical across cores)
    grp_meta = []  # [g][c] -> (idx_off_cols, T_cg, [(b_local, start, stop), ...])
    # start/stop bookkeeping per (g, b): first/last (c, t) in stream order
    off_slots = 0
    for g in range(N_GRP):
        blocks = _grp_blocks(g)
        first_seen = {}
        last_seen = {}
        for c in range(N_CHUNKS):
            for b in blocks:
                for t in range(int(tiles_cb[c, b])):
                    if b not in first_seen:
                        first_seen[b] = (c, t)
                    last_seen[b] = (c, t)
        # one PSUM accumulation group per (g): zero-matmul opens it, the
        # globally-last tile closes it (stop=True)
        glast = None
        for c in range(N_CHUNKS):
            for b in blocks:
                for t in range(int(tiles_cb[c, b])):
                    glast = (c, b, t)
        per_c = []
        for c in range(N_CHUNKS):
            tl = []
            for b in blocks:
                for t in range(int(tiles_cb[c, b])):
                    sp = glast == (c, b, t)
                    tl.append((b - blocks[0], b, False, sp))
            # split into sub-units of <= T_SUB tiles (smaller SBUF tiles ->
            # deeper buffering + finer-grained gather/matmul pipelining)
            subs = []
            for s0 in range(0, len(tl), T_SUB):
                subs.append((off_slots + s0 * 128, tl[s0 : s0 + T_SUB]))
            per_c.append(subs)
            off_slots += len(tl) * 128
        grp_meta.append(per_c)
    assert off_slots == total_slots

    # per-core packed slot arrays
    idx_wrapped = np.zeros((P_CORES, 128, total_slots // 16), dtype=np.int16)
    cidx_all = np.zeros((P_CORES, 128, total_tiles), dtype=BF16)

    for pc in range(P_CORES):
        flat_idx = np.zeros(total_slots, dtype=np.int16)
        flat_col = np.full(total_slots, -1, dtype=np.int64)
        for g in range(N_GRP):
            blocks = _grp_blocks(g)
            for c in range(N_CHUNKS):
                off = grp_meta[g][c][0][0]
                # fill per block
                boff = off
                for b in blocks:
                    n_slots = int(tiles_cb[c, b]) * 128
                    i = (pc * N_CHUNKS + c) * N_BLK + b
                    s0, s1 = starts[i], ends[i]
                    n_e = s1 - s0
                    assert n_e <= n_slots
                    # sort slots by source row: near-ascending DMA gather
                    # addresses (HBM row-buffer locality); dst col rides along
                    so = np.argsort(srow_s[s0:s1], kind="stable")
                    flat_idx[boff : boff + n_e] = srow_s[s0:s1][so]
                    flat_col[boff : boff + n_e] = col_s[s0:s1][so]
                    boff += n_slots
        wr = flat_idx.reshape(total_slots // 16, 16).T  # [16, S/16]
        idx_wrapped[pc] = np.tile(wr, (8, 1))
        cidx_all[pc] = (
            flat_col.astype(np.float32).reshape(total_tiles, 128).T.astype(BF16)
        )

    # MLP weights / scales layouts
    W1sb = np.ascontiguousarray(
        W1.astype(BF16).reshape(4, 128, F_HID).transpose(1, 0, 2)
    )  # [128, 4, 256]
    W2sb = np.ascontiguousarray(
        W2.astype(BF16).reshape(2, 128, F_OUT).transpose(1, 0, 2)
    )  # [128, 2, 64]
    b1sb = np.ascontiguousarray(b1.astype(np.float32).reshape(2, 128).T)  # [128,2]
    b2sb = b2.astype(np.float32).reshape(F_OUT, 1)  # [64,1]

    temp64 = np.asarray(temp, dtype=np.float64)

    dinv_rows = np.zeros((P_CORES, 128, N_BLK), dtype=np.float32)
    dinvt0_rows = np.zeros((P_CORES, 128, N_BLK), dtype=np.float32)
    dinv2_rows = np.zeros((P_CORES, 128, N_BLK), dtype=np.float32)
    dinv2t_rows = np.zeros((P_CORES, 128, K_HOPS, N_BLK), dtype=np.float32)
    dinvinv_rows = np.zeros((P_CORES, 128, N_BLK), dtype=np.float32)
    xT = np.zeros((P_CORES, 128, 4, NSH), dtype=BF16)
    for pc in range(P_CORES):
        dloc = dinv[pc * NSH : (pc + 1) * NSH]
        d2loc = dinv2[pc * NSH : (pc + 1) * NSH]

        def rows(vals):
            r = np.zeros(NSH_PAD, dtype=np.float32)
            r[:NSH] = vals
            return r.reshape(N_BLK, 128).T

        dinv_rows[pc] = rows(dloc)
        dinvt0_rows[pc] = rows(dloc * np.float32(temp64[0]))
        dinv2_rows[pc] = rows(d2loc)
        for k in range(K_HOPS):
            dinv2t_rows[pc, :, k, :] = rows(d2loc * np.float32(temp64[k + 1]))
        dinvinv_rows[pc] = rows(1.0 / dloc)
        xs = x[pc * NSH : (pc + 1) * NSH].astype(BF16)  # [12500, 512]
        xT[pc] = xs.T.reshape(4, 128, NSH).transpose(1, 0, 2)

    ident64 = np.eye(F_OUT, dtype=BF16)
    iota128 = np.tile(np.arange(128, dtype=np.float32).astype(BF16)[None, :], (128, 1))

    T_max = 0
    for g in range(N_GRP):
        for c in range(N_CHUNKS):
            for off, tl in grp_meta[g][c]:
                T_max = max(T_max, len(tl))

    meta = dict(
        tiles_cb=tiles_cb,
        grp_meta=grp_meta,
        total_slots=total_slots,
        total_tiles=total_tiles,
        T_max=T_max,
    )
    per_core = []
    for pc in range(P_CORES):
        per_core.append(
            {
                "xT": np.ascontiguousarray(xT[pc]),
                "W1sb": W1sb,
                "W2sb": W2sb,
                "b1sb": b1sb,
                "b2sb": b2sb,
                "idxs": np.ascontiguousarray(idx_wrapped[pc]),
                "cidx": np.ascontiguousarray(cidx_all[pc]),
                "iota128": iota128,
                "dinvt0_rows": np.ascontiguousarray(dinvt0_rows[pc]),
                "dinv_rows": np.ascontiguousarray(dinv_rows[pc]),
                "dinv2_rows": np.ascontiguousarray(dinv2_rows[pc]),
                "dinv2t_rows": np.ascontiguousarray(dinv2t_rows[pc]),
                "dinvinv_rows": np.ascontiguousarray(dinvinv_rows[pc]),
                "ident64": ident64,
            }
        )
    return meta, per_core


def _build_bass(meta, n_hops=K_HOPS):
    from concourse import bacc, bass, tile
    from concourse import mybir

    dt = mybir.dt
    AF = mybir.ActivationFunctionType
    ALU = mybir.AluOpType

    grp_meta = meta["grp_meta"]
    total_slots = meta["total_slots"]
    total_tiles = meta["total_tiles"]
    T_max = meta["T_max"]

    nc = bacc.Bacc(None, target_bir_lowering=False, num_swdge_queues=4)

    # ---- dram I/O ----
    xT_d = nc.dram_tensor("xT", [128, 4, NSH], dt.bfloat16, kind="ExternalInput")
    W1_d = nc.dram_tensor("W1sb", [128, 4, F_HID], dt.bfloat16, kind="ExternalInput")
    W2_d = nc.dram_tensor("W2sb", [128, 2, F_OUT], dt.bfloat16, kind="ExternalInput")
    b1_d = nc.dram_tensor("b1sb", [128, 2], dt.float32, kind="ExternalInput")
    b2_d = nc.dram_tensor("b2sb", [F_OUT, 1], dt.float32, kind="ExternalInput")
    idx_d = nc.dram_tensor(
        "idxs", [128, total_slots // 16], dt.int16, kind="ExternalInput"
    )
    cidx_d = nc.dram_tensor("cidx", [128, total_tiles], dt.bfloat16, kind="ExternalInput")
    iota_d = nc.dram_tensor("iota128", [128, 128], dt.bfloat16, kind="ExternalInput")
    dinvt0_d = nc.dram_tensor("dinvt0_rows", [128, N_BLK], dt.float32, kind="ExternalInput")
    dinvr_d = nc.dram_tensor("dinv_rows", [128, N_BLK], dt.float32, kind="ExternalInput")
    dinv2r_d = nc.dram_tensor("dinv2_rows", [128, N_BLK], dt.float32, kind="ExternalInput")
    dinv2t_d = nc.dram_tensor(
        "dinv2t_rows", [128, K_HOPS, N_BLK], dt.float32, kind="ExternalInput"
    )
    dinvinvr_d = nc.dram_tensor(
        "dinvinv_rows", [128, N_BLK], dt.float32, kind="ExternalInput"
    )
    id64_d = nc.dram_tensor("ident64", [F_OUT, F_OUT], dt.bfloat16, kind="ExternalInput")
    out_d = nc.dram_tensor("out", [NSH, F_OUT], dt.float32, kind="ExternalOutput")

    # persistent internal dram: double-buffered full u + AG inputs
    u_full = [
        nc.dram_tensor(
            f"u_full{i}",
            [P_CORES * NSH_PAD, 128],
            dt.bfloat16,
            kind="Internal",
            addr_space="Shared",
        )
        for i in range(2)
    ]
    u_in = [
        nc.dram_tensor(f"u_in{i}", [NSH_PAD, 128], dt.bfloat16, kind="Internal")
        for i in range(2)
    ]

    from concourse.library_config import mlp as _mlp_lib

    rg = [list(range(P_CORES))]

    with tile.TileContext(nc) as tc:
        nc.gpsimd.load_library(_mlp_lib)
        with (
            tc.tile_pool(name="const", bufs=1) as constp,
            tc.tile_pool(name="vbuf", bufs=1) as vbufp,
            tc.tile_pool(name="stage", bufs=1) as stagep,
            tc.tile_pool(name="mlp", bufs=3) as mlpp,
            tc.tile_pool(name="gat", bufs=8) as gatp,
            tc.tile_pool(name="idxp", bufs=8) as idxp,
            tc.tile_pool(name="mpool", bufs=6) as mp,
            tc.tile_pool(name="evac", bufs=4) as evacp,
            tc.tile_pool(name="psmlp", bufs=1, space="PSUM") as psmlp,
            tc.tile_pool(name="ps2", bufs=1, space="PSUM") as ps2p,
            tc.tile_pool(name="psT", bufs=1, space="PSUM") as psTp,
            tc.tile_pool(name="psg", bufs=4, space="PSUM") as psgp,
        ):
            # ---- constants resident in SBUF ----
            W1sb = constp.tile([128, 4, F_HID], dt.bfloat16)
            W2sb = constp.tile([128, 2, F_OUT], dt.bfloat16)
            b1sb = constp.tile([128, 2], dt.float32)
            b2sb = constp.tile([F_OUT, 1], dt.float32)
            dinvt0 = constp.tile([128, N_BLK], dt.float32)
            dinvr = constp.tile([128, N_BLK], dt.float32)
            dinv2r = constp.tile([128, N_BLK], dt.float32)
            dinv2t = constp.tile([128, K_HOPS, N_BLK], dt.float32)
            dinvinvr = constp.tile([128, N_BLK], dt.float32)
            id64 = constp.tile([F_OUT, F_OUT], dt.bfloat16)
            iota = constp.tile([128, 128], dt.bfloat16)
            zero_c = constp.tile([1, 128], dt.bfloat16)
            zero_r = constp.tile([1, BLK_PER_GRP * F_OUT], dt.bfloat16)
            for t_, d_ in [
                (W1sb, W1_d), (W2sb, W2_d), (b1sb, b1_d), (b2sb, b2_d),
                (dinvt0, dinvt0_d), (dinvr, dinvr_d), (dinv2r, dinv2r_d),
                (dinv2t, dinv2t_d), (dinvinvr, dinvinvr_d),
                (id64, id64_d), (iota, iota_d),
            ]:
                nc.sync.dma_start(t_[:], d_[:])
            nc.vector.memset(zero_c[:], 0.0)
            nc.vector.memset(zero_r[:], 0.0)

            # v accumulator [128, N_BLK, 64] f32 (row space); u row staging
            v_sb = vbufp.tile([128, N_BLK, F_OUT], dt.float32)
            u_stage = stagep.tile([128, N_BLK, 128], dt.bfloat16)
            nc.gpsimd.memset(u_stage[:], 0.0)

            # ================= MLP + u0 + v0 =================
            for w in range(N_WIN):
                ncols = _win_cols(w)
                xt = mlpp.tile([128, 4, WIN], dt.bfloat16, tag="xt")
                nc.sync.dma_start(
                    xt[:, :, :ncols], xT_d[:, :, w * WIN : w * WIN + ncols]
                )
                psA = psmlp.tile([128, WIN], dt.float32, tag="psA")
                psB = psmlp.tile([128, WIN], dt.float32, tag="psB")
                for k in range(4):
                    nc.tensor.matmul(
                        psA[:, :ncols], W1sb[:, k, 0:128], xt[:, k, :ncols],
                        start=(k == 0), stop=(k == 3),
                    )
                for k in range(4):
                    nc.tensor.matmul(
                        psB[:, :ncols], W1sb[:, k, 128:256], xt[:, k, :ncols],
                        start=(k == 0), stop=(k == 3),
                    )
                h1a = mlpp.tile([128, WIN], dt.bfloat16, tag="h1a")
                h1b = mlpp.tile([128, WIN], dt.bfloat16, tag="h1b")
                nc.scalar.activation(h1a[:, :ncols], psA[:, :ncols], AF.Relu, bias=b1sb[:, 0:1])
                nc.scalar.activation(h1b[:, :ncols], psB[:, :ncols], AF.Relu, bias=b1sb[:, 1:2])
                ps2 = ps2p.tile([F_OUT, WIN], dt.float32, tag="ps2")
                nc.tensor.matmul(ps2[:, :ncols], W2sb[:, 0, :], h1a[:, :ncols], start=True, stop=False)
                nc.tensor.matmul(ps2[:, :ncols], W2sb[:, 1, :], h1b[:, :ncols], start=False, stop=True)
                # h2^T = ps2 + b2 (bias add on DVE; keeps ACT on Relu only)
                h2tb = mlpp.tile([F_OUT, WIN], dt.bfloat16, tag="h2tb")
                nc.vector.tensor_scalar(
                    h2tb[:, :ncols], ps2[:, :ncols], b2sb[:, 0:1], None, ALU.add
                )
                # transpose to rows; u0 = dinv*h2; v0 = temp0*dinv*h2
                nblk = (ncols + 127) // 128
                for bs in range(nblk):
                    cw = min(128, ncols - bs * 128)
                    b = w * 4 + bs
                    pst = psTp.tile([128, F_OUT], dt.bfloat16, tag="ptr")
                    nc.tensor.transpose(
                        pst[:cw, :], h2tb[:, bs * 128 : bs * 128 + cw], id64[:]
                    )
                    nc.vector.tensor_scalar(
                        u_stage[:cw, b, 0:F_OUT], pst[:cw, :],
                        dinvr[:cw, b : b + 1], None, ALU.mult,
                    )
                    nc.vector.tensor_scalar(
                        v_sb[:cw, b, :], pst[:cw, :],
                        dinvt0[:cw, b : b + 1], None, ALU.mult,
                    )

            # write u0 rows -> u_in[0], AllGather -> u_full[0]
            skip_ag = os.environ.get("GPRGNN_SKIP_AG", "0") == "1"
            skip_flush_env = os.environ.get("GPRGNN_SKIP_FLUSH", "0") == "1"

            def flush_u(parity):
                if skip_flush_env and parity != 0:
                    # timing ablation only: drop the whole hop-boundary chain
                    return
                nc.sync.dma_start(
                    u_in[parity].rearrange("(p b) f -> p b f", p=128)[:],
                    u_stage[:],
                )
                if skip_ag:
                    # timing ablation only: results are wrong
                    nc.sync.dma_start(
                        u_full[parity][0:NSH_PAD, :], u_in[parity][:]
                    )
                    return
                nc.gpsimd.collective_compute(
                    "AllGather",
                    mybir.AluOpType.bypass,
                    ins=[u_in[parity][:]],
                    outs=[u_full[parity][:]],
                    replica_groups=rg,
                )

            flush_u(0)

            # ================= propagation hops =================
            unit_ctr = [0]  # global gather counter: stable slot<->queue map
            mmcols = F_OUT
            for k in range(1, n_hops + 1):
                rd = (k - 1) % 2
                wr_p = k % 2
                for g in range(N_GRP):
                    blocks = _grp_blocks(g)
                    nb = len(blocks)
                    psg = psgp.tile([128, BLK_PER_GRP * F_OUT], dt.float32, tag="psg")
                    nc.tensor.matmul(
                        psg[:, : nb * F_OUT], zero_c[:, :], zero_r[:, : nb * F_OUT],
                        start=True, stop=False,
                    )
                    for c in range(N_CHUNKS):
                      for off_slots, tl in grp_meta[g][c]:
                        T_cg = len(tl)
                        if T_cg == 0:
                            continue
                        S_cg = T_cg * 128
                        idxt = idxp.tile([128, T_max * 8], dt.int16, tag="idxt")
                        nc.sync.dma_start(
                            idxt[:, : S_cg // 16],
                            idx_d[:, off_slots // 16 : (off_slots + S_cg) // 16],
                        )
                        cixt = mp.tile([128, T_max], dt.bfloat16, tag="cixt")
                        nc.sync.dma_start(
                            cixt[:, :T_cg],
                            cidx_d[:, off_slots // 128 : off_slots // 128 + T_cg],
                        )
                        gbuf = gatp.tile([128, T_max, 128], dt.bfloat16, tag="gbuf")
                        nc.gpsimd.dma_gather(
                            gbuf[:, :T_cg, :],
                            u_full[rd][c * CHUNK_ROWS : (c + 1) * CHUNK_ROWS, :],
                            idxt[:, : S_cg // 16],
                            num_idxs=S_cg,
                            num_idxs_reg=S_cg,
                            elem_size=128,
                            single_packet=False,
                            queue_num=unit_ctr[0] % 4,
                        )
                        unit_ctr[0] += 1
                        mt = mp.tile([128, T_max, 128], dt.float8e4, tag="mt")
                        nc.vector.tensor_tensor(
                            mt[:, :T_cg, :],
                            iota[:].unsqueeze(1).broadcast_to([128, T_cg, 128]),
                            cixt[:, :T_cg].unsqueeze(2).broadcast_to([128, T_cg, 128]),
                            ALU.is_equal,
                        )
                        for t, (bl, b, st, sp) in enumerate(tl):
                            nc.tensor.matmul(
                                psg[:, bl * F_OUT : bl * F_OUT + mmcols],
                                mt[:, t, :],
                                gbuf[:, t, 0:mmcols],
                                start=st,
                                stop=sp,
                            )
                    # evacuate group: u rows (ACT) + v accumulate (DVE)
                    for j, b in enumerate(blocks):
                        cw = _blk_cols(b)
                        nc.scalar.activation(
                            u_stage[:cw, b, 0:F_OUT],
                            psg[:cw, j * F_OUT : (j + 1) * F_OUT],
                            AF.Identity,
                            scale=dinv2r[:cw, b : b + 1],
                        )
                        nc.vector.scalar_tensor_tensor(
                            v_sb[:cw, b, :],
                            psg[:cw, j * F_OUT : (j + 1) * F_OUT],
                            dinv2t[:cw, k - 1, b : b + 1],
                            v_sb[:cw, b, :],
                            ALU.mult,
                            ALU.add,
                        )
                if k < n_hops:
                    flush_u(wr_p)

            # ============ final: hidden = v / dinv ; log_softmax ============
            # Phased to avoid ACT table thrash: all Exp together, one Ln.
            nmxs = vbufp.tile([128, N_BLK], dt.float32)
            sumexs = vbufp.tile([128, N_BLK], dt.float32)
            lnss = vbufp.tile([128, N_BLK], dt.float32)
            nc.vector.memset(nmxs[:], 0.0)
            nc.vector.memset(sumexs[:], 1.0)
            for b in range(N_BLK):
                cw = _blk_cols(b)
                # hidden in place: v_sb *= 1/dinv
                nc.vector.tensor_scalar(
                    v_sb[:cw, b, :], v_sb[:cw, b, :],
                    dinvinvr[:cw, b : b + 1], None, ALU.mult,
                )
                # -max via reduce then negate fused: reduce to nmxs col
                nc.vector.tensor_reduce(
                    nmxs[:cw, b : b + 1], v_sb[:cw, b, :], mybir.AxisListType.X, ALU.max
                )
            nc.vector.tensor_scalar(nmxs[:], nmxs[:], -1.0, None, ALU.mult)
            for b in range(N_BLK):
                cw = _blk_cols(b)
                ex = evacp.tile([128, F_OUT], dt.float32, tag="ex")
                nc.scalar.activation(
                    ex[:cw, :], v_sb[:cw, b, :], AF.Exp, bias=nmxs[:cw, b : b + 1],
                    accum_out=sumexs[:cw, b : b + 1],
                )
            nc.scalar.activation(lnss[:], sumexs[:], AF.Ln)
            # nmxs <- nmxs - ln(sumex)
            nc.vector.tensor_tensor(nmxs[:], nmxs[:], lnss[:], ALU.subtract)
            for b in range(N_BLK):
                cw = _blk_cols(b)
                nc.vector.tensor_scalar(
                    v_sb[:cw, b, :], v_sb[:cw, b, :], nmxs[:cw, b : b + 1], None, ALU.add
                )
            # batched output DMA (blocks 0..96 full, block 97 has 84 rows)
            nc.sync.dma_start(
                out_d[: 97 * 128, :].rearrange("(b p) f -> p b f", p=128),
                v_sb[:, 0:97, :],
            )
            nc.sync.dma_start(
                out_d[97 * 128 :, :].rearrange("(b p) f -> p b f", p=84),
                v_sb[:84, 97:98, :],
            )

    nc.compile()
    return nc


def _kernel_numpy(x, edge_index, W1, b1, W2, b2, temp, n_hops=K_HOPS):
    x = np.asarray(x, np.float32)
    h = np.maximum(x @ W1 + b1, 0.0) @ W2 + b2
    src = np.concatenate([np.asarray(edge_index[0], np.int64), np.arange(N)])
    dst = np.concatenate([np.asarray(edge_index[1], np.int64), np.arange(N)])
    deg = np.bincount(dst, minlength=N).astype(np.float32)
    dinv = 1.0 / np.sqrt(deg)
    norm = (dinv[src] * dinv[dst]).astype(np.float32)
    hidden = h * np.float32(np.asarray(temp)[0])
    try:
        import scipy.sparse as sp

        A = sp.csr_matrix((norm, (dst, src)), shape=(N, N), dtype=np.float32)
        for k in range(n_hops):
            h = A @ h
            hidden = hidden + np.float32(np.asarray(temp)[k + 1]) * h
    except ImportError:
        for k in range(n_hops):
            nh = np.zeros_like(h)
            np.add.at(nh, dst, norm[:, None] * h[src])
            h = nh
            hidden = hidden + np.float32(np.asarray(temp)[k + 1]) * h
    m = hidden.max(axis=1, keepdims=True)
    e = np.exp(hidden - m)
    return (hidden - m - np.log(e.sum(axis=1, keepdims=True))).astype(np.float32)


def kernel(x, edge_index, W1, b1, W2, b2, temp, _n_hops=K_HOPS, _trace=False):
    if os.environ.get("GPRGNN_DEVICE", "1") == "1":
        try:
            return _kernel_device(x, edge_index, W1, b1, W2, b2, temp, _n_hops, _trace)
        except Exception as e:
            import traceback

            traceback.print_exc()
            print(f"[kernel] device path failed ({type(e).__name__}); numpy fallback")
    kernel._last_exec_ns = None
    return _kernel_numpy(x, edge_index, W1, b1, W2, b2, temp, _n_hops)


def _run_pjrt_timed(nc, in_maps, n_cores, n_rep=3):
    """Multi-core PJRT execution (axon) with timed repeats.

    Same lowering as bass2jax.run_bass_via_pjrt, but keeps the jitted
    executable and pre-places device inputs so repeat calls measure
    dispatch + HW execution only (no host->device transfers).
    """
    import time

    import jax
    from jax.sharding import Mesh, NamedSharding, PartitionSpec
    from jax.experimental.shard_map import shard_map

    from concourse import bass2jax as b2j
    from concourse import mybir

    b2j.install_neuronx_cc_hook()
    assert nc.dbg_addr is None
    partition_name = nc.partition_id_tensor.name if nc.partition_id_tensor else None

    in_names, out_names, out_avals, zero_outs = [], [], [], []
    for alloc in nc.m.functions[0].allocations:
        if not isinstance(alloc, mybir.MemoryLocationSet):
            continue
        name = alloc.memorylocations[0].name
        if alloc.kind == "ExternalInput":
            if name != partition_name:
                in_names.append(name)
        elif alloc.kind == "ExternalOutput":
            out_names.append(name)
            shape = tuple(alloc.tensor_shape)
            dtype = mybir.dt.np(alloc.dtype)
            out_avals.append(jax.core.ShapedArray(shape, dtype))
            zero_outs.append(np.zeros(shape, dtype))
    n_params = len(in_names)
    n_outs = len(out_avals)
    in_names_full = in_names + out_names
    if partition_name is not None:
        in_names_full.append(partition_name)
    donate = tuple(range(n_params, n_params + n_outs))

    def _body(*args):
        operands = list(args)
        if partition_name is not None:
            operands.append(b2j.partition_id_tensor())
        outs = b2j._bass_exec_p.bind(
            *operands,
            out_avals=tuple(out_avals),
            in_names=tuple(in_names_full),
            out_names=tuple(out_names),
            lowering_input_output_aliases=(),
            sim_require_finite=True,
            sim_require_nnan=True,
            nc=nc,
        )
        return tuple(outs)

    devices = jax.devices()[:n_cores]
    mesh = Mesh(np.asarray(devices), ("core",))
    in_specs = (PartitionSpec("core"),) * (n_params + n_outs)
    out_specs = (PartitionSpec("core"),) * n_outs
    sharded = jax.jit(
        shard_map(
            _body, mesh=mesh, in_specs=in_specs, out_specs=out_specs, check_rep=False
        ),
        donate_argnums=donate,
        keep_unused=True,
    )
    per_core_in = [[np.asarray(m[name]) for name in in_names] for m in in_maps]
    sharding = NamedSharding(mesh, PartitionSpec("core"))
    concat_in = [
        jax.device_put(
            np.concatenate([per_core_in[c][i] for c in range(n_cores)], axis=0),
            sharding,
        )
        for i in range(n_params)
    ]

    def fresh_zeros():
        return [
            jax.device_put(
                np.zeros((n_cores * z.shape[0], *z.shape[1:]), z.dtype), sharding
            )
            for z in zero_outs
        ]

    zero_sets = [fresh_zeros() for _ in range(n_rep + 1)]
    out_arrs = sharded(*concat_in, *zero_sets[0])  # compile + first run
    jax.block_until_ready(out_arrs)
    np_outs = [np.asarray(a) for a in out_arrs]
    times = []
    for r in range(n_rep):
        t0 = time.perf_counter()
        o = sharded(*concat_in, *zero_sets[r + 1])
        jax.block_until_ready(o)
        times.append(time.perf_counter() - t0)
        del o
    exec_ns = int(min(times) * 1e9)
    results = [
        {
            name: np_outs[i].reshape(n_cores, *out_avals[i].shape)[c]
            for i, name in enumerate(out_names)
        }
        for c in range(n_cores)
    ]
    return results, exec_ns, times


def _kernel_device(x, edge_index, W1, b1, W2, b2, temp, _n_hops=K_HOPS, _trace=False):
    meta, per_core = _prep_host(
        np.asarray(x), np.asarray(edge_index), np.asarray(W1), np.asarray(b1),
        np.asarray(W2), np.asarray(b2), np.asarray(temp),
    )
    nc = _build_bass(meta, n_hops=_n_hops)
    results, exec_ns, times = _run_pjrt_timed(nc, per_core, P_CORES)
    print(f"[kernel] timed runs: {['%.3fms' % (t*1e3) for t in times]}")
    outs = [results[c]["out"].astype(np.float32) for c in range(P_CORES)]
    full = np.concatenate(outs, axis=0)
    kernel._last_exec_ns = exec_ns
    kernel._last_res = None
    return full

